# revision 7
# baseline (speedup 1.0000x reference)
"""Trainium2 Bass kernel for nn_MemoryGraphBackprop (GNN message passing).

Strategy
--------
T=64 sequential steps over state [BS=2, N=1024, D=64].  The recurrence is
latency-bound and all operands (dense adjacency A = 4MB, cc signals = 2MB,
state = 1MB) fit in SBUF, while an 8-core shard would need a per-step
all-gather of pm (>=5us collective floor x 64 steps) that dwarfs the compute.
So: ONE NeuronCore, fully SBUF-resident recurrence.

Math (per step t):
    r   = A @ pm  (+ cc_t into nodes < C)          # PE, layout-2 psum
    dt  = decay * (1 - eot[b,t])  = F_t * D
    h'  = dt*h + (1-dt)*r
    pm' = tanh(prim * h')
With u := prim*h the update is
    u'  = F_t*D * u + W2_t * r,   W2_t = prim - F_t*(D*prim)
F_t is per-(b,*) -> per-PARTITION in layout-2 ([(b,d) part, n free]) so all
eot handling is tensor_scalar; D, prim, D*prim are constant tiles.

Engines: PE does 16 accumulating FD=512 float32r matmuls + the cc inject
(identity matmul) + 8 transposes of u' back to layout-1; ACT fuses tanh into
the PSUM->SBUF copy of the transposed u' (producing pm in layout-1 = next
step's lhsT); DVE/GPSIMD split the elementwise chain.

Layouts:
  l2 (state u, psum r):  [128 part = b*64+d, 1024 free = n]
  l1 (pm, matmul lhsT):  [128 part = n%128, free = (n//128)*128 + b*64 + d]
"""

import sys

if "/opt/trn_rl_repo" not in sys.path:
    sys.path.insert(0, "/opt/trn_rl_repo")

import numpy as np

import concourse.bass as bass
import concourse.mybir as mybir
import concourse.tile as tile
from concourse import bass_utils

BS, T, C, D = 2, 64, 64, 64
N = 1024
NT = N // 128  # 8 node chunks
P = 128        # BS*D partitions in layout-2

F32 = mybir.dt.float32
F32R = mybir.dt.float32r

# ---------------------------------------------------------------------------
# Workaround: this container's walrus accepts only ONE sync-wait on the SP
# Drain (TPB_CTRL_NO_STRUCT); Tile's tail drain attaches one wait per live
# semaphore.  Split the waits across multiple drain instructions.
# ---------------------------------------------------------------------------
from concourse.vector_clock import ScopedClock  # noqa: E402


def _patched_drain_and_barrier(self, tick_clock, wait_clock):
    drain_inst = self.nc.sync.drain()
    wait_clock.add_sem_waits(
        drain_inst.ins, ScopedClock({None: tick_clock.global_clock})
    )
    si = drain_inst.ins.sync_info
    if si is not None and si.on_wait is not None and len(si.on_wait) > 1:
        waits = list(si.on_wait)
        drain_inst.ins.sync_info = mybir.SyncInfo(
            on_wait=[waits[0]], on_update=si.on_update
        )
        for w in waits[1:]:
            d2 = self.nc.sync.drain()
            d2.ins.sync_info = mybir.SyncInfo(on_wait=[w], on_update=[])

    self.nc.all_engine_barrier()
    assert self.sems is not None
    popped = self.nc._tile_sem_poison_stack.pop()
    assert popped is self._sem_poison
    self.nc.clear_and_free_semaphores(list(self.sems.allocated().values()))
    self.nc.all_engine_barrier()


tile.TileContext._drain_and_barrier = _patched_drain_and_barrier


def _split_multi_waits(nc):
    """This walrus accepts only one sync-wait per instruction.  Hoist extra
    waits onto standalone InstEventSemaphore carriers on the same engine,
    inserted immediately before the instruction (program order preserved)."""
    n_carriers = 0
    for bb in nc.m.functions[0].blocks:
        insts = list(bb.instructions)
        out = []
        changed = False
        for inst in insts:
            si = inst.sync_info
            if si is not None and si.on_wait is not None and len(si.on_wait) > 1:
                waits = list(si.on_wait)
                for w in waits[:-1]:
                    n_carriers += 1
                    carrier = mybir.InstEventSemaphore(
                        name=f"waitsplit-{n_carriers}", ins=[], outs=[]
                    )
                    carrier.engine = inst.engine
                    carrier.sync_info = mybir.SyncInfo(on_wait=[w], on_update=[])
                    out.append(carrier)
                inst.sync_info = mybir.SyncInfo(
                    on_wait=[waits[-1]], on_update=si.on_update
                )
                changed = True
            out.append(inst)
        if changed:
            bb.instructions = out
    return n_carriers


# ---------------------------------------------------------------------------
# Host-side input massaging (layouts, scatter into dense A, norms, sigmoid).
# ---------------------------------------------------------------------------
def _prep_host(inputs):
    cc = np.asarray(inputs["cc_signals"], dtype=np.float32)       # [B,T,C,D]
    eot = np.asarray(inputs["eot_mask"]).astype(bool)             # [B,T]
    idx = np.asarray(inputs["conn_indices"]).astype(np.int64)     # [N,K]
    cmask = np.asarray(inputs["conn_mask"]).astype(np.float32)    # [N,K]
    prim = np.asarray(inputs["primitives"], dtype=np.float32)     # [N,D]
    w = np.asarray(inputs["conn_weights"], dtype=np.float32)      # [N,K]
    dlog = np.asarray(inputs["decay_logit"], dtype=np.float32)    # [N]
    h0 = np.asarray(inputs["h0"], dtype=np.float32)               # [B,N,D]
    pm0 = np.asarray(inputs["prev_msg0"], dtype=np.float32)       # [B,N,D]

    # dense adjacency, transposed for the layout-2 matmul (rhs[m, n] = A[n, m])
    A = np.zeros((N, N), dtype=np.float32)
    np.add.at(A, (np.arange(N)[:, None], idx), w * cmask)
    At = np.ascontiguousarray(A.T)                                # [m, n]
    at_host = At.reshape(NT, 128, N).transpose(1, 0, 2).reshape(128, NT * N)

    # L2-normalized cc in layout-1 slabs: [c, t*128 + b*64 + d]
    nrm = np.maximum(np.linalg.norm(cc, axis=-1, keepdims=True), 1e-8)
    ccn = (cc / nrm).astype(np.float32)
    cc_host = np.ascontiguousarray(
        ccn.transpose(2, 1, 0, 3).reshape(C, T * P)
    )

    decay = (1.0 / (1.0 + np.exp(-dlog.astype(np.float64)))).astype(np.float32)
    dec_l2 = np.ascontiguousarray(np.broadcast_to(decay[None, :], (P, N)))
    prim_l2 = np.ascontiguousarray(np.tile(prim.T, (BS, 1)))      # [128, N]
    dp_l2 = np.ascontiguousarray(prim_l2 * decay[None, :])        # D*prim

    fmat = np.repeat((~eot).astype(np.float32), D, axis=0)        # [128, T]
    fmat = np.ascontiguousarray(fmat)

    h0_l2 = h0.transpose(0, 2, 1).reshape(P, N)                   # [b*64+d, n]
    u0 = np.ascontiguousarray(prim_l2 * h0_l2)

    pm0_l1 = np.ascontiguousarray(
        pm0.reshape(BS, NT, 128, D).transpose(2, 1, 0, 3).reshape(128, NT * P)
    )

    return {
        "at": at_host,
        "cc": cc_host,
        "prim": prim_l2,
        "dp": dp_l2,
        "dec": dec_l2,
        "fmat": fmat,
        "id64": np.eye(64, dtype=np.float32),
        "id128": np.eye(128, dtype=np.float32),
        "u0": u0,
        "pm0": pm0_l1,
    }


# ---------------------------------------------------------------------------
# Device kernel
# ---------------------------------------------------------------------------
def _build_bass():
    nc = bass.Bass("TRN2", target_bir_lowering=False, debug=False)

    at_d = nc.dram_tensor("at", [128, NT * N], F32R, kind="ExternalInput")
    cc_d = nc.dram_tensor("cc", [C, T * P], F32, kind="ExternalInput")
    prim_d = nc.dram_tensor("prim", [P, N], F32, kind="ExternalInput")
    dp_d = nc.dram_tensor("dp", [P, N], F32, kind="ExternalInput")
    dec_d = nc.dram_tensor("dec", [P, N], F32, kind="ExternalInput")
    f_d = nc.dram_tensor("fmat", [P, T], F32, kind="ExternalInput")
    id64_d = nc.dram_tensor("id64", [64, 64], F32, kind="ExternalInput")
    id128_d = nc.dram_tensor("id128", [128, 128], F32, kind="ExternalInput")
    u0_d = nc.dram_tensor("u0", [P, N], F32, kind="ExternalInput")
    pm0_d = nc.dram_tensor("pm0", [128, NT * P], F32R, kind="ExternalInput")
    out_d = nc.dram_tensor("out", [T, C, P], F32, kind="ExternalOutput")

    Tanh = mybir.ActivationFunctionType.Tanh

    with tile.TileContext(nc) as tc:
        with (
            tc.tile_pool(name="consts", bufs=1) as consts,
            tc.tile_pool(name="state", bufs=2) as state,
            tc.tile_pool(name="tmp", bufs=2) as tmp,
            tc.tile_pool(name="ps0", bufs=2, space="PSUM") as ps0p,
            tc.tile_pool(name="ps1", bufs=2, space="PSUM") as ps1p,
            tc.tile_pool(name="pt", bufs=4, space="PSUM") as ptp,
        ):
            # --- load constants ---
            at_sb = consts.tile([128, NT * N], F32R)
            for m in range(NT):
                nc.sync.dma_start(
                    out=at_sb[:, m * N:(m + 1) * N],
                    in_=at_d.ap()[:, m * N:(m + 1) * N],
                )
            cc_sb = consts.tile([C, T * P], F32)
            for q in range(4):
                s = slice(q * (T * P) // 4, (q + 1) * (T * P) // 4)
                nc.sync.dma_start(out=cc_sb[:, s], in_=cc_d.ap()[:, s])
            prim_sb = consts.tile([P, N], F32)
            nc.sync.dma_start(out=prim_sb[:], in_=prim_d.ap()[:])
            dp_sb = consts.tile([P, N], F32)
            nc.sync.dma_start(out=dp_sb[:], in_=dp_d.ap()[:])
            dec_sb = consts.tile([P, N], F32)
            nc.sync.dma_start(out=dec_sb[:], in_=dec_d.ap()[:])
            f_sb = consts.tile([P, T], F32)
            nc.sync.dma_start(out=f_sb[:], in_=f_d.ap()[:])
            id64_sb = consts.tile([64, 64], F32)
            nc.sync.dma_start(out=id64_sb[:], in_=id64_d.ap()[:])
            id128_sb = consts.tile([128, 128], F32)
            nc.sync.dma_start(out=id128_sb[:], in_=id128_d.ap()[:])

            u = state.tile([P, N], F32, tag="u")
            nc.sync.dma_start(out=u[:], in_=u0_d.ap()[:])
            pm = state.tile([128, NT * P], F32R, tag="pm")
            nc.sync.dma_start(out=pm[:], in_=pm0_d.ap()[:])

            for t in range(T):
                ft = f_sb[:, t:t + 1]
                # ---- off-critical-path per-step tiles ----
                # w0 = F_t * D ;  s_b = u * w0  (= dt * u)
                w0 = tmp.tile([P, N], F32, tag="w0")
                nc.vector.tensor_scalar_mul(w0[:], dec_sb[:], ft)
                sb_t = tmp.tile([P, N], F32, tag="sb")
                nc.gpsimd.tensor_mul(sb_t[:], u[:], w0[:])
                # W2 = prim - F_t * (D*prim)   (= (1-dt)*prim)
                w1 = tmp.tile([P, N], F32, tag="w1")
                nc.vector.tensor_scalar_mul(w1[:], dp_sb[:], ft)
                w2 = tmp.tile([P, N], F32, tag="w2")
                nc.gpsimd.tensor_sub(w2[:], prim_sb[:], w1[:])

                # ---- matmuls: r = A @ pm (+ cc inject), layout-2 psum ----
                ps = [
                    ps0p.tile([P, 512], F32, tag="ps0", name="ps0"),
                    ps1p.tile([P, 512], F32, tag="ps1", name="ps1"),
                ]
                for m in range(NT):
                    lhsT = pm[:, m * P:(m + 1) * P]
                    for h in range(2):
                        nc.tensor.matmul(
                            ps[h][:],
                            lhsT,
                            at_sb[:, m * N + h * 512: m * N + (h + 1) * 512],
                            start=(m == 0),
                            stop=(m == NT - 1 and h == 1),
                        )
                # cc_t into nodes < C (psum half 0, cols 0:64)
                nc.tensor.matmul(
                    ps[0][:, 0:64],
                    cc_sb[:, t * P:(t + 1) * P],
                    id64_sb[:],
                    start=False,
                    stop=True,
                    skip_group_check=True,
                )

                # ---- on-chain: u' = W2 * r + s_b ----
                un = state.tile([P, N], F32, tag="u")
                for h in range(2):
                    sl = slice(h * 512, (h + 1) * 512)
                    x = tmp.tile([P, 512], F32, tag=f"x{h}")
                    nc.vector.tensor_mul(x[:], ps[h][:], w2[:, sl])
                    nc.vector.tensor_add(un[:, sl], x[:], sb_t[:, sl])

                # ---- transpose u' to layout-1, tanh into pm ----
                pmn = state.tile([128, NT * P], F32R, tag="pm")
                for h in range(2):
                    pt = ptp.tile([128, 512], F32, tag="pt")
                    for j in range(4):
                        m = h * 4 + j
                        nc.tensor.transpose(
                            pt[:, j * 128:(j + 1) * 128],
                            un[:, m * 128:(m + 1) * 128],
                            id128_sb[:],
                        )
                    nc.scalar.activation(
                        pmn[:, h * 512:(h + 1) * 512], pt[:], Tanh
                    )

                # ---- emit output slice: nodes < C, layout-1 chunk 0 ----
                nc.sync.dma_start(
                    out=out_d.ap()[t], in_=pmn[0:C, 0:P].bitcast(F32)
                )

                u, pm = un, pmn

    _split_multi_waits(nc)
    return nc


RUN_KWARGS: dict = {}
_BUILT = None


def _get_built():
    global _BUILT
    if _BUILT is None:
        _BUILT = _build_bass()
    return _BUILT


def kernel(**inputs) -> np.ndarray:
    host = _prep_host(inputs)
    nc = _get_built()
    res = bass_utils.run_bass_kernel_spmd(nc, [host], core_ids=[0], **RUN_KWARGS)
    kernel.last_result = res
    out_dev = res.results[0]["out"]                               # [T, C, 128]
    out = out_dev.reshape(T, C, BS, D).transpose(2, 0, 1, 3)      # [B,T,C,D]
    return np.ascontiguousarray(out)


if __name__ == "__main__":
    rng = np.random.default_rng(0)
    print("standalone smoke: building bass module...")
    _get_built()
    print("built ok")


# revision 10
# speedup vs baseline: 1.6746x; 1.6746x over previous
"""Trainium2 Bass kernel for nn_MemoryGraphBackprop (GNN message passing).

Strategy
--------
T=64 sequential steps over state [BS=2, N=1024, D=64].  The recurrence is
latency-bound and all operands (dense adjacency A = 4MB, cc signals = 2MB,
state = 1MB) fit in SBUF, while an 8-core shard would need a per-step
all-gather of pm (>=5us collective floor x 64 steps) that dwarfs the compute.
So: ONE NeuronCore, fully SBUF-resident recurrence.

Math (per step t):
    r   = A @ pm  (+ cc_t into nodes < C)          # PE, layout-2 psum
    dt  = decay * (1 - eot[b,t])  = F_t * D
    h'  = dt*h + (1-dt)*r
    pm' = tanh(prim * h')
With u := prim*h the update is
    u'  = F_t*D * u + W2_t * r,   W2_t = prim - F_t*(D*prim)
F_t is per-(b,*) -> per-PARTITION in layout-2 ([(b,d) part, n free]) so all
eot handling is tensor_scalar; D, prim, D*prim are constant tiles.

Engines: PE does 16 accumulating FD=512 float32r matmuls + the cc inject
(identity matmul) + 8 transposes of u' back to layout-1; ACT fuses tanh into
the PSUM->SBUF copy of the transposed u' (producing pm in layout-1 = next
step's lhsT); DVE/GPSIMD split the elementwise chain.

Layouts:
  l2 (state u, psum r):  [128 part = b*64+d, 1024 free = n]
  l1 (pm, matmul lhsT):  [128 part = n%128, free = (n//128)*128 + b*64 + d]
"""

import sys

if "/opt/trn_rl_repo" not in sys.path:
    sys.path.insert(0, "/opt/trn_rl_repo")

import numpy as np

import concourse.bass as bass
import concourse.mybir as mybir
import concourse.tile as tile
from concourse import bass_utils

BS, T, C, D = 2, 64, 64, 64
N = 1024
NT = N // 128  # 8 node chunks
P = 128        # BS*D partitions in layout-2

F32 = mybir.dt.float32
F32R = mybir.dt.float32r

# ---------------------------------------------------------------------------
# Workaround: this container's walrus accepts only ONE sync-wait on the SP
# Drain (TPB_CTRL_NO_STRUCT); Tile's tail drain attaches one wait per live
# semaphore.  Split the waits across multiple drain instructions.
# ---------------------------------------------------------------------------
from concourse.vector_clock import ScopedClock  # noqa: E402


def _patched_drain_and_barrier(self, tick_clock, wait_clock):
    drain_inst = self.nc.sync.drain()
    wait_clock.add_sem_waits(
        drain_inst.ins, ScopedClock({None: tick_clock.global_clock})
    )
    si = drain_inst.ins.sync_info
    if si is not None and si.on_wait is not None and len(si.on_wait) > 1:
        waits = list(si.on_wait)
        drain_inst.ins.sync_info = mybir.SyncInfo(
            on_wait=[waits[0]], on_update=si.on_update
        )
        for w in waits[1:]:
            d2 = self.nc.sync.drain()
            d2.ins.sync_info = mybir.SyncInfo(on_wait=[w], on_update=[])

    self.nc.all_engine_barrier()
    assert self.sems is not None
    popped = self.nc._tile_sem_poison_stack.pop()
    assert popped is self._sem_poison
    self.nc.clear_and_free_semaphores(list(self.sems.allocated().values()))
    self.nc.all_engine_barrier()


tile.TileContext._drain_and_barrier = _patched_drain_and_barrier


def _split_multi_waits(nc):
    """This walrus accepts only one sync-wait per instruction.  Hoist extra
    waits onto standalone InstEventSemaphore carriers on the same engine,
    inserted immediately before the instruction (program order preserved)."""
    n_carriers = 0
    for bb in nc.m.functions[0].blocks:
        insts = list(bb.instructions)
        out = []
        changed = False
        for inst in insts:
            si = inst.sync_info
            if si is not None and si.on_wait is not None and len(si.on_wait) > 1:
                waits = list(si.on_wait)
                for w in waits[:-1]:
                    n_carriers += 1
                    carrier = mybir.InstEventSemaphore(
                        name=f"waitsplit-{n_carriers}", ins=[], outs=[]
                    )
                    carrier.engine = inst.engine
                    carrier.sync_info = mybir.SyncInfo(on_wait=[w], on_update=[])
                    out.append(carrier)
                inst.sync_info = mybir.SyncInfo(
                    on_wait=[waits[-1]], on_update=si.on_update
                )
                changed = True
            out.append(inst)
        if changed:
            bb.instructions = out
    return n_carriers


# ---------------------------------------------------------------------------
# Host-side input massaging (layouts, scatter into dense A, norms, sigmoid).
# ---------------------------------------------------------------------------
def _prep_host(inputs):
    cc = np.asarray(inputs["cc_signals"], dtype=np.float32)       # [B,T,C,D]
    eot = np.asarray(inputs["eot_mask"]).astype(bool)             # [B,T]
    idx = np.asarray(inputs["conn_indices"]).astype(np.int64)     # [N,K]
    cmask = np.asarray(inputs["conn_mask"]).astype(np.float32)    # [N,K]
    prim = np.asarray(inputs["primitives"], dtype=np.float32)     # [N,D]
    w = np.asarray(inputs["conn_weights"], dtype=np.float32)      # [N,K]
    dlog = np.asarray(inputs["decay_logit"], dtype=np.float32)    # [N]
    h0 = np.asarray(inputs["h0"], dtype=np.float32)               # [B,N,D]
    pm0 = np.asarray(inputs["prev_msg0"], dtype=np.float32)       # [B,N,D]

    # dense adjacency, transposed for the layout-2 matmul (rhs[m, n] = A[n, m])
    A = np.zeros((N, N), dtype=np.float32)
    np.add.at(A, (np.arange(N)[:, None], idx), w * cmask)
    At = np.ascontiguousarray(A.T)                                # [m, n]
    at_host = At.reshape(NT, 128, N).transpose(1, 0, 2).reshape(128, NT * N)

    # L2-normalized cc in layout-1 slabs: [c, t*128 + b*64 + d]
    nrm = np.maximum(np.linalg.norm(cc, axis=-1, keepdims=True), 1e-8)
    ccn = (cc / nrm).astype(np.float32)
    cc_host = np.ascontiguousarray(
        ccn.transpose(2, 1, 0, 3).reshape(C, T * P)
    )

    decay = (1.0 / (1.0 + np.exp(-dlog.astype(np.float64)))).astype(np.float32)
    dec_l2 = np.ascontiguousarray(np.broadcast_to(decay[None, :], (P, N)))
    prim_l2 = np.ascontiguousarray(np.tile(prim.T, (BS, 1)))      # [128, N]
    dp_l2 = np.ascontiguousarray(prim_l2 * decay[None, :])        # D*prim

    fmat = np.repeat((~eot).astype(np.float32), D, axis=0)        # [128, T]
    fmat = np.ascontiguousarray(fmat)

    h0_l2 = h0.transpose(0, 2, 1).reshape(P, N)                   # [b*64+d, n]
    u0 = np.ascontiguousarray(prim_l2 * h0_l2)

    pm0_l1 = np.ascontiguousarray(
        pm0.reshape(BS, NT, 128, D).transpose(2, 1, 0, 3).reshape(128, NT * P)
    )

    import ml_dtypes

    bf16 = ml_dtypes.bfloat16
    return {
        "at": at_host.astype(bf16),
        "cc": cc_host.astype(bf16),
        "prim": prim_l2.astype(bf16),
        "dp": dp_l2.astype(bf16),
        "dec": dec_l2.astype(bf16),
        "fmat": fmat,
        "id64": np.eye(64, dtype=bf16),
        "id128": np.eye(128, dtype=bf16),
        "u0": u0.astype(bf16),
        "pm0": pm0_l1.astype(bf16),
    }


# ---------------------------------------------------------------------------
# Device kernel
# ---------------------------------------------------------------------------
def _build_bass():
    nc = bass.Bass("TRN2", target_bir_lowering=False, debug=False)

    BF = mybir.dt.bfloat16

    at_d = nc.dram_tensor("at", [128, NT * N], BF, kind="ExternalInput")
    cc_d = nc.dram_tensor("cc", [C, T * P], BF, kind="ExternalInput")
    prim_d = nc.dram_tensor("prim", [P, N], BF, kind="ExternalInput")
    dp_d = nc.dram_tensor("dp", [P, N], BF, kind="ExternalInput")
    dec_d = nc.dram_tensor("dec", [P, N], BF, kind="ExternalInput")
    f_d = nc.dram_tensor("fmat", [P, T], F32, kind="ExternalInput")
    id64_d = nc.dram_tensor("id64", [64, 64], BF, kind="ExternalInput")
    id128_d = nc.dram_tensor("id128", [128, 128], BF, kind="ExternalInput")
    u0_d = nc.dram_tensor("u0", [P, N], BF, kind="ExternalInput")
    pm0_d = nc.dram_tensor("pm0", [128, NT * P], BF, kind="ExternalInput")
    out_d = nc.dram_tensor("out", [T, C, P], F32, kind="ExternalOutput")

    Tanh = mybir.ActivationFunctionType.Tanh
    Copy = mybir.ActivationFunctionType.Copy

    with tile.TileContext(nc) as tc:
        with (
            tc.tile_pool(name="consts", bufs=1) as consts,
            tc.tile_pool(name="state", bufs=2) as state,
            tc.tile_pool(name="tmp", bufs=2) as tmp,
            tc.tile_pool(name="ps0", bufs=2, space="PSUM") as ps0p,
            tc.tile_pool(name="ps1", bufs=2, space="PSUM") as ps1p,
            tc.tile_pool(name="pt", bufs=4, space="PSUM") as ptp,
        ):
            # --- load constants ---
            at_sb = consts.tile([128, NT * N], BF)
            for m in range(NT):
                nc.sync.dma_start(
                    out=at_sb[:, m * N:(m + 1) * N],
                    in_=at_d.ap()[:, m * N:(m + 1) * N],
                )
            cc_sb = consts.tile([C, T * P], BF)
            for q in range(4):
                s = slice(q * (T * P) // 4, (q + 1) * (T * P) // 4)
                nc.sync.dma_start(out=cc_sb[:, s], in_=cc_d.ap()[:, s])
            prim_sb = consts.tile([P, N], BF)
            nc.sync.dma_start(out=prim_sb[:], in_=prim_d.ap()[:])
            dp_sb = consts.tile([P, N], BF)
            nc.sync.dma_start(out=dp_sb[:], in_=dp_d.ap()[:])
            dec_sb = consts.tile([P, N], BF)
            nc.sync.dma_start(out=dec_sb[:], in_=dec_d.ap()[:])
            f_sb = consts.tile([P, T], F32)
            nc.sync.dma_start(out=f_sb[:], in_=f_d.ap()[:])
            id64_sb = consts.tile([64, 64], BF)
            nc.sync.dma_start(out=id64_sb[:], in_=id64_d.ap()[:])
            id128_sb = consts.tile([128, 128], BF)
            nc.sync.dma_start(out=id128_sb[:], in_=id128_d.ap()[:])

            u = state.tile([P, N], BF, tag="u")
            nc.sync.dma_start(out=u[:], in_=u0_d.ap()[:])
            pm = state.tile([128, NT * P], BF, tag="pm")
            nc.sync.dma_start(out=pm[:], in_=pm0_d.ap()[:])

            for t in range(T):
                ft = f_sb[:, t:t + 1]
                # ---- off-critical-path per-step tiles ----
                # w0 = F_t * D  (ACT: copy with per-partition scale)
                w0 = tmp.tile([P, N], BF, tag="w0")
                nc.scalar.activation(w0[:], dec_sb[:], Copy, scale=ft)
                # s_b = u * w0  (= dt * u)
                sb_t = tmp.tile([P, N], BF, tag="sb")
                nc.vector.tensor_mul(sb_t[:], u[:], w0[:])
                # W2 = prim - F_t * (D*prim)   (= (1-dt)*prim)
                w1 = tmp.tile([P, N], BF, tag="w1")
                nc.vector.tensor_scalar_mul(w1[:], dp_sb[:], ft)
                w2 = tmp.tile([P, N], BF, tag="w2")
                nc.vector.tensor_sub(w2[:], prim_sb[:], w1[:])

                # ---- matmuls: r = A @ pm (+ cc inject), layout-2 psum ----
                ps = [
                    ps0p.tile([P, 512], F32, tag="ps0", name="ps0"),
                    ps1p.tile([P, 512], F32, tag="ps1", name="ps1"),
                ]
                for m in range(NT):
                    lhsT = pm[:, m * P:(m + 1) * P]
                    for h in range(2):
                        nc.tensor.matmul(
                            ps[h][:],
                            lhsT,
                            at_sb[:, m * N + h * 512: m * N + (h + 1) * 512],
                            start=(m == 0),
                            stop=(m == NT - 1 and h == 1),
                        )
                # cc_t into nodes < C (psum half 0, cols 0:64)
                nc.tensor.matmul(
                    ps[0][:, 0:64],
                    cc_sb[:, t * P:(t + 1) * P],
                    id64_sb[:],
                    start=False,
                    stop=True,
                    skip_group_check=True,
                )

                # ---- on-chain: u' = W2 * r + s_b ----
                un = state.tile([P, N], BF, tag="u")
                for h in range(2):
                    sl = slice(h * 512, (h + 1) * 512)
                    x = tmp.tile([P, 512], BF, tag=f"x{h}")
                    nc.vector.tensor_mul(x[:], ps[h][:], w2[:, sl])
                    nc.vector.tensor_add(un[:, sl], x[:], sb_t[:, sl])

                # ---- transpose u' to layout-1, tanh into pm ----
                pmn = state.tile([128, NT * P], BF, tag="pm")
                pts = []
                for h in range(2):
                    pt = ptp.tile([128, 512], BF, tag="pt", name="pt")
                    pts.append(pt)
                    for j in range(4):
                        m = h * 4 + j
                        nc.tensor.transpose(
                            pt[:, j * 128:(j + 1) * 128],
                            un[:, m * 128:(m + 1) * 128],
                            id128_sb[:],
                        )
                    nc.scalar.activation(
                        pmn[:, h * 512:(h + 1) * 512], pt[:], Tanh
                    )

                # ---- output slice (nodes < C) in full fp32 from psum ----
                out_sb = tmp.tile([C, P], F32, tag="out_sb")
                nc.scalar.activation(
                    out_sb[:], pts[0][0:C, 0:P], Tanh
                )
                nc.sync.dma_start(out=out_d.ap()[t], in_=out_sb[:])

                u, pm = un, pmn

    _split_multi_waits(nc)
    return nc


RUN_KWARGS: dict = {}
_BUILT = None


def _get_built():
    global _BUILT
    if _BUILT is None:
        _BUILT = _build_bass()
    return _BUILT


def kernel(**inputs) -> np.ndarray:
    host = _prep_host(inputs)
    nc = _get_built()
    res = bass_utils.run_bass_kernel_spmd(nc, [host], core_ids=[0], **RUN_KWARGS)
    kernel.last_result = res
    out_dev = res.results[0]["out"]                               # [T, C, 128]
    out = out_dev.reshape(T, C, BS, D).transpose(2, 0, 1, 3)      # [B,T,C,D]
    return np.ascontiguousarray(out)


if __name__ == "__main__":
    rng = np.random.default_rng(0)
    print("standalone smoke: building bass module...")
    _get_built()
    print("built ok")


# revision 11
# speedup vs baseline: 2.2050x; 1.3167x over previous
"""Trainium2 Bass kernel for nn_MemoryGraphBackprop (GNN message passing).

Strategy
--------
T=64 sequential steps over state [BS=2, N=1024, D=64].  The recurrence is
latency-bound and all operands (dense adjacency A = 4MB, cc signals = 2MB,
state = 1MB) fit in SBUF, while an 8-core shard would need a per-step
all-gather of pm (>=5us collective floor x 64 steps) that dwarfs the compute.
So: ONE NeuronCore, fully SBUF-resident recurrence.

Math (per step t):
    r   = A @ pm  (+ cc_t into nodes < C)          # PE, layout-2 psum
    dt  = decay * (1 - eot[b,t])  = F_t * D
    h'  = dt*h + (1-dt)*r
    pm' = tanh(prim * h')
With u := prim*h the update is
    u'  = F_t*D * u + W2_t * r,   W2_t = prim - F_t*(D*prim)
F_t is per-(b,*) -> per-PARTITION in layout-2 ([(b,d) part, n free]) so all
eot handling is tensor_scalar; D, prim, D*prim are constant tiles.

Engines: PE does 16 accumulating FD=512 float32r matmuls + the cc inject
(identity matmul) + 8 transposes of u' back to layout-1; ACT fuses tanh into
the PSUM->SBUF copy of the transposed u' (producing pm in layout-1 = next
step's lhsT); DVE/GPSIMD split the elementwise chain.

Layouts:
  l2 (state u, psum r):  [128 part = b*64+d, 1024 free = n]
  l1 (pm, matmul lhsT):  [128 part = n%128, free = (n//128)*128 + b*64 + d]
"""

import sys

if "/opt/trn_rl_repo" not in sys.path:
    sys.path.insert(0, "/opt/trn_rl_repo")

import numpy as np

import concourse.bass as bass
import concourse.mybir as mybir
import concourse.tile as tile
from concourse import bass_utils

BS, T, C, D = 2, 64, 64, 64
N = 1024
NT = N // 128  # 8 node chunks
P = 128        # BS*D partitions in layout-2

F32 = mybir.dt.float32
F32R = mybir.dt.float32r

# ---------------------------------------------------------------------------
# Workaround: this container's walrus accepts only ONE sync-wait on the SP
# Drain (TPB_CTRL_NO_STRUCT); Tile's tail drain attaches one wait per live
# semaphore.  Split the waits across multiple drain instructions.
# ---------------------------------------------------------------------------
from concourse.vector_clock import ScopedClock  # noqa: E402


def _patched_drain_and_barrier(self, tick_clock, wait_clock):
    drain_inst = self.nc.sync.drain()
    wait_clock.add_sem_waits(
        drain_inst.ins, ScopedClock({None: tick_clock.global_clock})
    )
    si = drain_inst.ins.sync_info
    if si is not None and si.on_wait is not None and len(si.on_wait) > 1:
        waits = list(si.on_wait)
        drain_inst.ins.sync_info = mybir.SyncInfo(
            on_wait=[waits[0]], on_update=si.on_update
        )
        for w in waits[1:]:
            d2 = self.nc.sync.drain()
            d2.ins.sync_info = mybir.SyncInfo(on_wait=[w], on_update=[])

    self.nc.all_engine_barrier()
    assert self.sems is not None
    popped = self.nc._tile_sem_poison_stack.pop()
    assert popped is self._sem_poison
    self.nc.clear_and_free_semaphores(list(self.sems.allocated().values()))
    self.nc.all_engine_barrier()


tile.TileContext._drain_and_barrier = _patched_drain_and_barrier


def _split_multi_waits(nc):
    """This walrus accepts only one sync-wait per instruction.  Hoist extra
    waits onto standalone InstEventSemaphore carriers on the same engine,
    inserted immediately before the instruction (program order preserved)."""
    n_carriers = 0
    for bb in nc.m.functions[0].blocks:
        insts = list(bb.instructions)
        out = []
        changed = False
        for inst in insts:
            si = inst.sync_info
            if si is not None and si.on_wait is not None and len(si.on_wait) > 1:
                waits = list(si.on_wait)
                for w in waits[:-1]:
                    n_carriers += 1
                    carrier = mybir.InstEventSemaphore(
                        name=f"waitsplit-{n_carriers}", ins=[], outs=[]
                    )
                    carrier.engine = inst.engine
                    carrier.sync_info = mybir.SyncInfo(on_wait=[w], on_update=[])
                    out.append(carrier)
                inst.sync_info = mybir.SyncInfo(
                    on_wait=[waits[-1]], on_update=si.on_update
                )
                changed = True
            out.append(inst)
        if changed:
            bb.instructions = out
    return n_carriers


# ---------------------------------------------------------------------------
# Host-side input massaging (layouts, scatter into dense A, norms, sigmoid).
# ---------------------------------------------------------------------------
def _prep_host(inputs):
    cc = np.asarray(inputs["cc_signals"], dtype=np.float32)       # [B,T,C,D]
    eot = np.asarray(inputs["eot_mask"]).astype(bool)             # [B,T]
    idx = np.asarray(inputs["conn_indices"]).astype(np.int64)     # [N,K]
    cmask = np.asarray(inputs["conn_mask"]).astype(np.float32)    # [N,K]
    prim = np.asarray(inputs["primitives"], dtype=np.float32)     # [N,D]
    w = np.asarray(inputs["conn_weights"], dtype=np.float32)      # [N,K]
    dlog = np.asarray(inputs["decay_logit"], dtype=np.float32)    # [N]
    h0 = np.asarray(inputs["h0"], dtype=np.float32)               # [B,N,D]
    pm0 = np.asarray(inputs["prev_msg0"], dtype=np.float32)       # [B,N,D]

    # dense adjacency, transposed for the layout-2 matmul (rhs[m, n] = A[n, m])
    A = np.zeros((N, N), dtype=np.float32)
    np.add.at(A, (np.arange(N)[:, None], idx), w * cmask)
    At = np.ascontiguousarray(A.T)                                # [m, n]
    at_host = At.reshape(NT, 128, N).transpose(1, 0, 2).reshape(128, NT * N)

    # L2-normalized cc in layout-1 slabs: [c, t*128 + b*64 + d]
    nrm = np.maximum(np.linalg.norm(cc, axis=-1, keepdims=True), 1e-8)
    ccn = (cc / nrm).astype(np.float32)
    cc_host = np.ascontiguousarray(
        ccn.transpose(2, 1, 0, 3).reshape(C, T * P)
    )

    decay = (1.0 / (1.0 + np.exp(-dlog.astype(np.float64)))).astype(np.float32)
    dec_l2 = np.ascontiguousarray(np.broadcast_to(decay[None, :], (P, N)))
    prim_l2 = np.ascontiguousarray(np.tile(prim.T, (BS, 1)))      # [128, N]
    dp_l2 = np.ascontiguousarray(prim_l2 * decay[None, :])        # D*prim

    fmat = np.repeat((~eot).astype(np.float32), D, axis=0)        # [128, T]
    fmat = np.ascontiguousarray(fmat)

    h0_l2 = h0.transpose(0, 2, 1).reshape(P, N)                   # [b*64+d, n]
    u0 = np.ascontiguousarray(prim_l2 * h0_l2)

    pm0_l1 = np.ascontiguousarray(
        pm0.reshape(BS, NT, 128, D).transpose(2, 1, 0, 3).reshape(128, NT * P)
    )

    import ml_dtypes

    bf16 = ml_dtypes.bfloat16
    return {
        "at": at_host.astype(bf16),
        "cc": cc_host.astype(bf16),
        "prim": prim_l2.astype(bf16),
        "dp": dp_l2.astype(bf16),
        "dec": dec_l2.astype(bf16),
        "fmat": fmat,
        "id64": np.eye(64, dtype=bf16),
        "id128": np.eye(128, dtype=bf16),
        "u0": u0.astype(bf16),
        "pm0": pm0_l1.astype(bf16),
    }


# ---------------------------------------------------------------------------
# Device kernel
# ---------------------------------------------------------------------------
def _build_bass():
    nc = bass.Bass("TRN2", target_bir_lowering=False, debug=False)

    BF = mybir.dt.bfloat16

    at_d = nc.dram_tensor("at", [128, NT * N], BF, kind="ExternalInput")
    cc_d = nc.dram_tensor("cc", [C, T * P], BF, kind="ExternalInput")
    prim_d = nc.dram_tensor("prim", [P, N], BF, kind="ExternalInput")
    dp_d = nc.dram_tensor("dp", [P, N], BF, kind="ExternalInput")
    dec_d = nc.dram_tensor("dec", [P, N], BF, kind="ExternalInput")
    f_d = nc.dram_tensor("fmat", [P, T], F32, kind="ExternalInput")
    id64_d = nc.dram_tensor("id64", [64, 64], BF, kind="ExternalInput")
    id128_d = nc.dram_tensor("id128", [128, 128], BF, kind="ExternalInput")
    u0_d = nc.dram_tensor("u0", [P, N], BF, kind="ExternalInput")
    pm0_d = nc.dram_tensor("pm0", [128, NT * P], BF, kind="ExternalInput")
    out_d = nc.dram_tensor("out", [T, C, P], F32, kind="ExternalOutput")

    Tanh = mybir.ActivationFunctionType.Tanh
    Copy = mybir.ActivationFunctionType.Copy

    with tile.TileContext(nc) as tc:
        with (
            tc.tile_pool(name="consts", bufs=1) as consts,
            tc.tile_pool(name="state", bufs=2) as state,
            tc.tile_pool(name="tmp", bufs=2) as tmp,
            tc.tile_pool(name="ps0", bufs=2, space="PSUM") as ps0p,
            tc.tile_pool(name="ps1", bufs=2, space="PSUM") as ps1p,
            tc.tile_pool(name="pt", bufs=4, space="PSUM") as ptp,
        ):
            # --- load constants ---
            at_sb = consts.tile([128, NT * N], BF)
            for m in range(NT):
                nc.sync.dma_start(
                    out=at_sb[:, m * N:(m + 1) * N],
                    in_=at_d.ap()[:, m * N:(m + 1) * N],
                )
            cc_sb = consts.tile([C, T * P], BF)
            for q in range(4):
                s = slice(q * (T * P) // 4, (q + 1) * (T * P) // 4)
                nc.sync.dma_start(out=cc_sb[:, s], in_=cc_d.ap()[:, s])
            prim_sb = consts.tile([P, N], BF)
            nc.sync.dma_start(out=prim_sb[:], in_=prim_d.ap()[:])
            dp_sb = consts.tile([P, N], BF)
            nc.sync.dma_start(out=dp_sb[:], in_=dp_d.ap()[:])
            dec_sb = consts.tile([P, N], BF)
            nc.sync.dma_start(out=dec_sb[:], in_=dec_d.ap()[:])
            f_sb = consts.tile([P, T], F32)
            nc.sync.dma_start(out=f_sb[:], in_=f_d.ap()[:])
            id64_sb = consts.tile([64, 64], BF)
            nc.sync.dma_start(out=id64_sb[:], in_=id64_d.ap()[:])
            id128_sb = consts.tile([128, 128], BF)
            nc.sync.dma_start(out=id128_sb[:], in_=id128_d.ap()[:])

            u = state.tile([P, N], BF, tag="u")
            nc.sync.dma_start(out=u[:], in_=u0_d.ap()[:])
            pm = state.tile([128, NT * P], BF, tag="pm")
            nc.sync.dma_start(out=pm[:], in_=pm0_d.ap()[:])

            for t in range(T):
                ft = f_sb[:, t:t + 1]
                # ---- off-critical-path per-step tiles ----
                # w0 = F_t * D  (ACT: copy with per-partition scale)
                w0 = tmp.tile([P, N], BF, tag="w0")
                nc.scalar.activation(w0[:], dec_sb[:], Copy, scale=ft)
                # s_b = u * w0  (= dt * u)
                sb_t = tmp.tile([P, N], BF, tag="sb")
                nc.vector.tensor_mul(sb_t[:], u[:], w0[:])
                # W2 = prim - F_t * (D*prim)   (= (1-dt)*prim)
                w1 = tmp.tile([P, N], BF, tag="w1")
                nc.vector.tensor_scalar_mul(w1[:], dp_sb[:], ft)
                w2 = tmp.tile([P, N], BF, tag="w2")
                nc.vector.tensor_sub(w2[:], prim_sb[:], w1[:])

                # ---- matmuls: r = A @ pm (+ cc inject), layout-2 psum ----
                # All half-0 matmuls first, then half-1: half-0's elementwise
                # chain + transposes + tanh overlap half-1's matmuls on PE.
                ps = [
                    ps0p.tile([P, 512], F32, tag="ps0", name="ps0"),
                    ps1p.tile([P, 512], F32, tag="ps1", name="ps1"),
                ]
                un = state.tile([P, N], BF, tag="u")
                pmn = state.tile([128, NT * P], BF, tag="pm")
                pts = [
                    ptp.tile([128, 512], BF, tag="pt", name="pt"),
                    ptp.tile([128, 512], BF, tag="pt", name="pt"),
                ]
                for h in range(2):
                    if h == 0:
                        # cc_t into nodes < C (psum half 0, cols 0:64)
                        nc.tensor.matmul(
                            ps[0][:, 0:64],
                            cc_sb[:, t * P:(t + 1) * P],
                            id64_sb[:],
                            start=True,
                            stop=False,
                            skip_group_check=True,
                        )
                    for m in range(NT):
                        nc.tensor.matmul(
                            ps[h][:],
                            pm[:, m * P:(m + 1) * P],
                            at_sb[:, m * N + h * 512: m * N + (h + 1) * 512],
                            start=(h == 1 and m == 0),
                            stop=(m == NT - 1),
                            skip_group_check=True,
                        )

                    # ---- on-chain: u'_h = W2_h * r_h + s_b_h ----
                    sl = slice(h * 512, (h + 1) * 512)
                    x = tmp.tile([P, 512], BF, tag=f"x{h}", name="x")
                    nc.vector.tensor_mul(x[:], ps[h][:], w2[:, sl])
                    nc.vector.tensor_add(un[:, sl], x[:], sb_t[:, sl])

                    # ---- transpose u'_h to layout-1, tanh into pm ----
                    for j in range(4):
                        m = h * 4 + j
                        nc.tensor.transpose(
                            pts[h][:, j * 128:(j + 1) * 128],
                            un[:, m * 128:(m + 1) * 128],
                            id128_sb[:],
                        )
                    nc.scalar.activation(
                        pmn[:, h * 512:(h + 1) * 512], pts[h][:], Tanh
                    )

                # ---- output slice (nodes < C) in full fp32 from psum ----
                out_sb = tmp.tile([C, P], F32, tag="out_sb")
                nc.scalar.activation(
                    out_sb[:], pts[0][0:C, 0:P], Tanh
                )
                nc.sync.dma_start(out=out_d.ap()[t], in_=out_sb[:])

                u, pm = un, pmn

    _split_multi_waits(nc)
    return nc


RUN_KWARGS: dict = {}
_BUILT = None


def _get_built():
    global _BUILT
    if _BUILT is None:
        _BUILT = _build_bass()
    return _BUILT


def kernel(**inputs) -> np.ndarray:
    host = _prep_host(inputs)
    nc = _get_built()
    res = bass_utils.run_bass_kernel_spmd(nc, [host], core_ids=[0], **RUN_KWARGS)
    kernel.last_result = res
    out_dev = res.results[0]["out"]                               # [T, C, 128]
    out = out_dev.reshape(T, C, BS, D).transpose(2, 0, 1, 3)      # [B,T,C,D]
    return np.ascontiguousarray(out)


if __name__ == "__main__":
    rng = np.random.default_rng(0)
    print("standalone smoke: building bass module...")
    _get_built()
    print("built ok")
